# revision 23
# baseline (speedup 1.0000x reference)
"""Data-parallel 3x3 conv2d (stride 1, pad 1) on 8 Trainium2 NeuronCores.

Problem: x [32, 64, 112, 112] f32, weight [128, 64, 3, 3] f32, bias [128]
-> out [32, 128, 112, 112] f32.

Sharding: batch N=32 split 4 images per core across 8 cores; weight/bias
replicated (forward only, no collectives needed).

v3 design (bf16 datapath, 5 matmul rounds per tile):
  - Host packs x into the padded SBUF layout in bf16: per image a
    [64, 116*114] buffer whose flat[0:13000] slice is xpad rows 0..113
    ("A" layout) and flat[114:13114] the same shifted one padded row
    ("B").  Loads are 2 fat contiguous DMAs per image (A -> partitions
    0-63, B -> 64-127): no staging, half the HBM bytes of f32.
  - A second SBUF tile L2 = [A ; C=xpad<<1col] is built on-chip: lower
    half is a DVE copy of A, upper half a cross-partition SBUF->SBUF
    DMA of A shifted one column.
  - Implicit GEMM, 5 full K=128 matmul rounds per 4-row PSUM tile,
    448 moving cols each (3D AP skips the 2 pad cols per row):
      s0..s2: xt @ f+kw      -> taps (0,kw) via A + (1,kw) via B
      s3:     L2 @ f+2*114   -> taps (2,0) via A + (2,1) via C
      s4:     L2 @ f+2*114+2 -> tap  (2,2) via A (upper weights zero)
    (vs 6 rounds for the naive pairing; the 4.5-round tile_position
    variant with concurrent same-bank accumulation hangs TRN2, and
    K=64 row-group matmuls to different banks run serially, so 5
    sequential full rounds is the fastest correct schedule measured.)
  - PSUM f32 accumulate; ScalarE activation(Identity, bias) drops the 2
    pad columns per row and writes bf16; batched contiguous DMA stores.
  - Output returned as f32 after a host-side upcast.  End-to-end rel
    err ~3e-3 (bf16 inputs/weights/output, f32 accumulate).
"""
import sys

if '/opt/trn_rl_repo' not in sys.path:
    sys.path.insert(0, '/opt/trn_rl_repo')

import numpy as np

N, CIN, HH, WW = 32, 64, 112, 112
OC = 128
NCORES = 8
N_PER_CORE = N // NCORES

HP = WP = HH + 2           # 114 padded
HROWS = 116                # host rows per image (2 extra zero rows)
FLAT = HROWS * WP          # 13224 host flat size
XLEN = 13000               # SBUF half length (12996 + AP overrun slack)
RPT = 4                    # output rows per PSUM tile
NCOL = RPT * WP            # 456 moving columns per matmul
NT = HH // RPT             # 28 tiles per image

_cache = {}


def _build():
    import concourse.bacc as bacc
    import concourse.mybir as mybir
    from concourse.tile import TileContext

    F32 = mybir.dt.float32
    BF16 = mybir.dt.bfloat16

    nc = bacc.Bacc("TRN2", target_bir_lowering=False, debug=False,
                   num_devices=NCORES)
    x = nc.declare_dram_parameter("x", [N_PER_CORE, CIN, FLAT], BF16,
                                  isOutput=False)
    wt = nc.declare_dram_parameter("wt", [128, 5 * 128], BF16, isOutput=False)
    bias = nc.declare_dram_parameter("bias", [128, 1], F32, isOutput=False)
    y = nc.declare_dram_parameter("y", [N_PER_CORE, OC, HH, WW], BF16,
                                  isOutput=True)
    xa = x.ap()
    ya = y.ap()

    with TileContext(nc) as tc:
        with (
            tc.tile_pool(name="wpool", bufs=1) as wpool,
            tc.tile_pool(name="xpool", bufs=1) as xpool,
            tc.tile_pool(name="opool", bufs=6) as opool,
            tc.tile_pool(name="pspool", bufs=8, space="PSUM") as pspool,
        ):
            wtile = wpool.tile([128, 5 * 128], BF16, tag="w")
            nc.sync.dma_start(out=wtile[:, :], in_=wt[:, :])
            btile = wpool.tile([128, 1], F32, tag="b")
            nc.sync.dma_start(out=btile[:, :], in_=bias[:, :])

            NBUF = 3
            xts = [xpool.tile([128, XLEN], BF16, tag=f"x{i}", name=f"xt{i}")
                   for i in range(NBUF)]
            l2s = [xpool.tile([128, XLEN], BF16, tag=f"l{i}", name=f"l2{i}")
                   for i in range(NBUF)]
            def load_image(n, nch=4):
                # chunked loads: compute tile t only depends on the chunks
                # covering its rows; image 0 uses finer chunks so the first
                # matmul can start earlier
                offs = [XLEN * c // nch for c in range(nch)] + [XLEN]
                xt, l2 = xts[n % NBUF], l2s[n % NBUF]
                for c in range(nch):
                    o0, o1 = offs[c], offs[c + 1]
                    nc.sync.dma_start(out=xt[0:64, o0:o1],
                                      in_=xa[n, :, o0:o1])
                    nc.sync.dma_start(out=xt[64:128, o0:o1],
                                      in_=xa[n, :, WP + o0:WP + o1])
                    # L2 upper = xpad shifted one column -- also a contiguous
                    # HBM slice (offset +1), so load it directly: no
                    # dependency on the A-half DMA
                    d0 = max(0, o0 - 1)
                    nc.sync.dma_start(out=l2[64:128, d0:o1 - 1],
                                      in_=xa[n, :, d0 + 1:o1])
                    # L2 lower = A: DVE copy in steady state (even-side DMA
                    # engines are the input bottleneck); direct HBM load for
                    # image 0 to shorten the startup dependency chain
                    if n == 0:
                        nc.sync.dma_start(out=l2[0:64, o0:o1],
                                          in_=xa[n, :, o0:o1])
                    else:
                        nc.vector.tensor_copy(l2[0:64, o0:o1],
                                              xt[0:64, o0:o1])

            def compute_image(n, batch=4):
                xt, l2 = xts[n % NBUF], l2s[n % NBUF]
                ot = None
                def mm3(ps, wcol, src, off, start, stop):
                    # moving operand skips the 2 pad columns per row via a
                    # 3D AP: 448 streamed cols/matmul instead of 456
                    rhs = src[:, off: off + NCOL].rearrange(
                        "c (r t) -> c r t", r=RPT, t=WP)[:, :, 0:WW]
                    out = ps[:, 0:RPT * WW].rearrange(
                        "o (r t) -> o r t", r=RPT, t=WW)
                    nc.tensor.matmul(
                        out, wtile[:, wcol * 128:(wcol + 1) * 128], rhs,
                        start=start, stop=stop)

                for t in range(NT):
                    f0 = t * RPT * WP
                    # bank-sized psum tile; matmuls write the first 448 cols
                    ps = pspool.tile([128, 512], F32, tag="ps")
                    for s in range(3):
                        mm3(ps, s, xt, f0 + s, s == 0, False)
                    mm3(ps, 3, l2, f0 + 2 * WP, False, False)
                    mm3(ps, 4, l2, f0 + 2 * WP + 2, False, True)
                    if t % batch == 0:
                        ot = opool.tile([128, batch * RPT * WW], BF16,
                                        tag="o")
                    half = (t % batch) * RPT * WW
                    nc.scalar.activation(
                        ot[:, half:half + RPT * WW], ps[:, 0:RPT * WW],
                        mybir.ActivationFunctionType.Identity,
                        bias=btile[:, :])
                    if t % batch == batch - 1:
                        yflat = ya[n, :, :, :].rearrange("o h w -> o (h w)")
                        nc.scalar.dma_start(
                            out=yflat[:, (t - batch + 1) * RPT * WW:
                                      (t + 1) * RPT * WW],
                            in_=ot[:, 0:batch * RPT * WW])

            # dep-free warm-up matmuls run while the first image loads, so
            # the PE HAM clock-gate reaches 8/8 before the first real matmul.
            # They read a memset tile (no DMA dependency) so they start as
            # soon as the engines come up rather than after the weight load.
            warm = wpool.tile([128, 640], BF16, tag="warm")
            nc.gpsimd.memset(warm[:, :], 0.0)
            for _ in range(10):
                psw = pspool.tile([128, 512], F32, tag="ps", name="psw")
                nc.tensor.matmul(psw[:, :], warm[:, 0:128],
                                 warm[:, 128:640], start=True, stop=True)

            load_image(0, nch=8)
            load_image(1)
            for n in range(N_PER_CORE):
                if n + 2 < N_PER_CORE:
                    load_image(n + 2)
                # finer store batching on the last image shortens the drain
                compute_image(n, batch=4 if n + 1 < N_PER_CORE else 1)
    nc.compile()
    return nc


def _pack_weights(weight: np.ndarray):
    """[O=128, C=64, 3, 3] -> [128, 5*128] bf16 slab layout.

    cols 0-383: slabs s=kw: rows 0-63 = w[:, :, 0, kw].T (A half),
                rows 64-127 = w[:, :, 1, kw].T (B half)
    cols 384-511: pair slab: rows 0-63 = w[:, :, 2, 0].T (A),
                  rows 64-127 = w[:, :, 2, 1].T (C)
    cols 512-639: single slab: rows 0-63 = w[:, :, 2, 2].T, rows 64-127 = 0
    """
    import ml_dtypes
    w5 = np.zeros((5, 128, 128), np.float32)   # [slab, k, o]
    wt_ = weight.astype(np.float32).transpose(2, 3, 1, 0)  # [kh, kw, c, o]
    for kw in range(3):
        w5[kw, 0:64] = wt_[0, kw]
        w5[kw, 64:128] = wt_[1, kw]
    w5[3, 0:64] = wt_[2, 0]
    w5[3, 64:128] = wt_[2, 1]
    w5[4, 0:64] = wt_[2, 2]
    out = w5.transpose(1, 0, 2).reshape(128, 5 * 128)
    return np.ascontiguousarray(out).astype(ml_dtypes.bfloat16)


def _pack_x(x: np.ndarray):
    """[N, 64, 112, 112] f32 -> [N, 64, 116*114] bf16 padded layout."""
    import ml_dtypes
    xp = np.zeros((N, CIN, HROWS, WP), np.float32)
    xp[:, :, 1:1 + HH, 1:1 + WW] = x
    return np.ascontiguousarray(
        xp.reshape(N, CIN, FLAT)).astype(ml_dtypes.bfloat16)


def kernel(x: np.ndarray, weight: np.ndarray, bias: np.ndarray,
           _trace: bool = False) -> np.ndarray:
    from concourse.bass_utils import run_bass_kernel_spmd

    x = np.asarray(x, dtype=np.float32)
    weight = np.asarray(weight, dtype=np.float32)
    bias = np.asarray(bias, dtype=np.float32)
    assert x.shape == (N, CIN, HH, WW), x.shape
    assert weight.shape == (OC, CIN, 3, 3), weight.shape
    assert bias.shape == (OC,), bias.shape

    if 'nc' not in _cache:
        _cache['nc'] = _build()
    nc = _cache['nc']

    xp = _pack_x(x)
    wtp = _pack_weights(weight)
    bp = np.ascontiguousarray(bias.reshape(128, 1))
    in_maps = [
        {"x": np.ascontiguousarray(xp[N_PER_CORE * i: N_PER_CORE * (i + 1)]),
         "wt": wtp, "bias": bp}
        for i in range(NCORES)
    ]
    res = run_bass_kernel_spmd(nc, in_maps, core_ids=list(range(NCORES)),
                               trace=_trace)
    out = np.concatenate([res.results[i]["y"] for i in range(NCORES)],
                         axis=0).astype(np.float32)
    if _trace:
        _cache['last_exec_time_ns'] = res.exec_time_ns
    return out


# revision 26
# speedup vs baseline: 1.0775x; 1.0775x over previous
"""Data-parallel 3x3 conv2d (stride 1, pad 1) on 8 Trainium2 NeuronCores.

Problem: x [32, 64, 112, 112] f32, weight [128, 64, 3, 3] f32, bias [128]
-> out [32, 128, 112, 112] f32.

Sharding: batch N=32 split 4 images per core across 8 cores; weight/bias
replicated (forward only, no collectives needed).

v3 design (bf16 datapath, 5 matmul rounds per tile):
  - Host packs x into the padded SBUF layout in bf16: per image a
    [64, 116*114] buffer whose flat[0:13000] slice is xpad rows 0..113
    ("A" layout) and flat[114:13114] the same shifted one padded row
    ("B").  Loads are 2 fat contiguous DMAs per image (A -> partitions
    0-63, B -> 64-127): no staging, half the HBM bytes of f32.
  - A second SBUF tile L2 = [A ; C=xpad<<1col] is built on-chip: lower
    half is a DVE copy of A, upper half a cross-partition SBUF->SBUF
    DMA of A shifted one column.
  - Implicit GEMM, 5 full K=128 matmul rounds per 4-row PSUM tile,
    448 moving cols each (3D AP skips the 2 pad cols per row):
      s0..s2: xt @ f+kw      -> taps (0,kw) via A + (1,kw) via B
      s3:     L2 @ f+2*114   -> taps (2,0) via A + (2,1) via C
      s4:     L2 @ f+2*114+2 -> tap  (2,2) via A (upper weights zero)
    (vs 6 rounds for the naive pairing; the 4.5-round tile_position
    variant with concurrent same-bank accumulation hangs TRN2, and
    K=64 row-group matmuls to different banks run serially, so 5
    sequential full rounds is the fastest correct schedule measured.)
  - PSUM f32 accumulate; ScalarE activation(Identity, bias) drops the 2
    pad columns per row and writes bf16; batched contiguous DMA stores.
  - Output returned as f32 after a host-side upcast.  End-to-end rel
    err ~3e-3 (bf16 inputs/weights/output, f32 accumulate).
"""
import sys

if '/opt/trn_rl_repo' not in sys.path:
    sys.path.insert(0, '/opt/trn_rl_repo')

import numpy as np

N, CIN, HH, WW = 32, 64, 112, 112
OC = 128
NCORES = 8
N_PER_CORE = N // NCORES

HP = WP = HH + 2           # 114 padded
HROWS = 116                # host rows per image (2 extra zero rows)
FLAT = HROWS * WP          # 13224 host flat size
XLEN = 13000               # SBUF half length (12996 + AP overrun slack)
RPT = 4                    # output rows per PSUM tile
NCOL = RPT * WP            # 456 moving columns per matmul
NT = HH // RPT             # 28 tiles per image

_cache = {}


def _build():
    import concourse.bacc as bacc
    import concourse.mybir as mybir
    from concourse.tile import TileContext

    F32 = mybir.dt.float32
    BF16 = mybir.dt.bfloat16

    nc = bacc.Bacc("TRN2", target_bir_lowering=False, debug=False,
                   num_devices=NCORES)
    x = nc.declare_dram_parameter("x", [N_PER_CORE, CIN, FLAT], BF16,
                                  isOutput=False)
    wt = nc.declare_dram_parameter("wt", [128, 5 * 128], BF16, isOutput=False)
    bias = nc.declare_dram_parameter("bias", [128, 1], F32, isOutput=False)
    y = nc.declare_dram_parameter("y", [N_PER_CORE, OC, HH, WW], BF16,
                                  isOutput=True)
    xa = x.ap()
    ya = y.ap()

    with TileContext(nc) as tc:
        with (
            tc.tile_pool(name="wpool", bufs=1) as wpool,
            tc.tile_pool(name="xpool", bufs=1) as xpool,
            tc.tile_pool(name="opool", bufs=6) as opool,
            tc.tile_pool(name="pspool", bufs=8, space="PSUM") as pspool,
        ):
            wtile = wpool.tile([128, 5 * 128], BF16, tag="w")
            nc.sync.dma_start(out=wtile[:, :], in_=wt[:, :])
            btile = wpool.tile([128, 1], F32, tag="b")
            nc.sync.dma_start(out=btile[:, :], in_=bias[:, :])

            NBUF = 3
            xts = [xpool.tile([128, XLEN], BF16, tag=f"x{i}", name=f"xt{i}")
                   for i in range(NBUF)]
            l2s = [xpool.tile([128, XLEN], BF16, tag=f"l{i}", name=f"l2{i}")
                   for i in range(NBUF)]
            def load_image(n, nch=4):
                # chunked loads: compute tile t only depends on the chunks
                # covering its rows; image 0 uses finer chunks so the first
                # matmul can start earlier
                offs = [XLEN * c // nch for c in range(nch)] + [XLEN]
                xt, l2 = xts[n % NBUF], l2s[n % NBUF]
                for c in range(nch):
                    o0, o1 = offs[c], offs[c + 1]
                    nc.sync.dma_start(out=xt[0:64, o0:o1],
                                      in_=xa[n, :, o0:o1])
                    nc.sync.dma_start(out=xt[64:128, o0:o1],
                                      in_=xa[n, :, WP + o0:WP + o1])
                    # L2 lower = A (straight DVE copy), upper = A shifted one
                    # column (cross-partition SBUF->SBUF DMA); the shifted
                    # chunk stays within xt chunk c via the -1 offset
                    nc.vector.tensor_copy(l2[0:64, o0:o1], xt[0:64, o0:o1])
                    d0 = max(0, o0 - 1)
                    nc.sync.dma_start(out=l2[64:128, d0:o1 - 1],
                                      in_=xt[0:64, d0 + 1:o1])

            def compute_image(n, batch=4):
                xt, l2 = xts[n % NBUF], l2s[n % NBUF]
                ot = None
                def mm3(ps, wcol, src, off, start, stop):
                    # moving operand skips the 2 pad columns per row via a
                    # 3D AP: 448 streamed cols/matmul instead of 456
                    rhs = src[:, off: off + NCOL].rearrange(
                        "c (r t) -> c r t", r=RPT, t=WP)[:, :, 0:WW]
                    out = ps[:, 0:RPT * WW].rearrange(
                        "o (r t) -> o r t", r=RPT, t=WW)
                    nc.tensor.matmul(
                        out, wtile[:, wcol * 128:(wcol + 1) * 128], rhs,
                        start=start, stop=stop)

                for t in range(NT):
                    f0 = t * RPT * WP
                    # bank-sized psum tile; matmuls write the first 448 cols
                    ps = pspool.tile([128, 512], F32, tag="ps")
                    for s in range(3):
                        mm3(ps, s, xt, f0 + s, s == 0, False)
                    mm3(ps, 3, l2, f0 + 2 * WP, False, False)
                    mm3(ps, 4, l2, f0 + 2 * WP + 2, False, True)
                    if t % batch == 0:
                        ot = opool.tile([128, batch * RPT * WW], BF16,
                                        tag="o")
                    half = (t % batch) * RPT * WW
                    nc.scalar.activation(
                        ot[:, half:half + RPT * WW], ps[:, 0:RPT * WW],
                        mybir.ActivationFunctionType.Identity,
                        bias=btile[:, :])
                    if t % batch == batch - 1:
                        yflat = ya[n, :, :, :].rearrange("o h w -> o (h w)")
                        nc.scalar.dma_start(
                            out=yflat[:, (t - batch + 1) * RPT * WW:
                                      (t + 1) * RPT * WW],
                            in_=ot[:, 0:batch * RPT * WW])

            # dep-free warm-up matmuls run while the first image loads, so
            # the PE HAM clock-gate reaches 8/8 before the first real matmul.
            # They read a memset tile (no DMA dependency) so they start as
            # soon as the engines come up rather than after the weight load.
            warm = wpool.tile([128, 640], BF16, tag="warm")
            nc.gpsimd.memset(warm[:, :], 0.0)
            for _ in range(10):
                psw = pspool.tile([128, 512], F32, tag="ps", name="psw")
                nc.tensor.matmul(psw[:, :], warm[:, 0:128],
                                 warm[:, 128:640], start=True, stop=True)

            load_image(0, nch=8)
            load_image(1)
            for n in range(N_PER_CORE):
                if n + 2 < N_PER_CORE:
                    load_image(n + 2)
                # finer store batching on the last image shortens the drain
                compute_image(n, batch=4 if n + 1 < N_PER_CORE else 2)
    nc.compile()
    return nc


def _pack_weights(weight: np.ndarray):
    """[O=128, C=64, 3, 3] -> [128, 5*128] bf16 slab layout.

    cols 0-383: slabs s=kw: rows 0-63 = w[:, :, 0, kw].T (A half),
                rows 64-127 = w[:, :, 1, kw].T (B half)
    cols 384-511: pair slab: rows 0-63 = w[:, :, 2, 0].T (A),
                  rows 64-127 = w[:, :, 2, 1].T (C)
    cols 512-639: single slab: rows 0-63 = w[:, :, 2, 2].T, rows 64-127 = 0
    """
    import ml_dtypes
    w5 = np.zeros((5, 128, 128), np.float32)   # [slab, k, o]
    wt_ = weight.astype(np.float32).transpose(2, 3, 1, 0)  # [kh, kw, c, o]
    for kw in range(3):
        w5[kw, 0:64] = wt_[0, kw]
        w5[kw, 64:128] = wt_[1, kw]
    w5[3, 0:64] = wt_[2, 0]
    w5[3, 64:128] = wt_[2, 1]
    w5[4, 0:64] = wt_[2, 2]
    out = w5.transpose(1, 0, 2).reshape(128, 5 * 128)
    return np.ascontiguousarray(out).astype(ml_dtypes.bfloat16)


def _pack_x(x: np.ndarray):
    """[N, 64, 112, 112] f32 -> [N, 64, 116*114] bf16 padded layout."""
    import ml_dtypes
    xp = np.zeros((N, CIN, HROWS, WP), np.float32)
    xp[:, :, 1:1 + HH, 1:1 + WW] = x
    return np.ascontiguousarray(
        xp.reshape(N, CIN, FLAT)).astype(ml_dtypes.bfloat16)


def kernel(x: np.ndarray, weight: np.ndarray, bias: np.ndarray,
           _trace: bool = False) -> np.ndarray:
    from concourse.bass_utils import run_bass_kernel_spmd

    x = np.asarray(x, dtype=np.float32)
    weight = np.asarray(weight, dtype=np.float32)
    bias = np.asarray(bias, dtype=np.float32)
    assert x.shape == (N, CIN, HH, WW), x.shape
    assert weight.shape == (OC, CIN, 3, 3), weight.shape
    assert bias.shape == (OC,), bias.shape

    if 'nc' not in _cache:
        _cache['nc'] = _build()
    nc = _cache['nc']

    xp = _pack_x(x)
    wtp = _pack_weights(weight)
    bp = np.ascontiguousarray(bias.reshape(128, 1))
    in_maps = [
        {"x": np.ascontiguousarray(xp[N_PER_CORE * i: N_PER_CORE * (i + 1)]),
         "wt": wtp, "bias": bp}
        for i in range(NCORES)
    ]
    res = run_bass_kernel_spmd(nc, in_maps, core_ids=list(range(NCORES)),
                               trace=_trace)
    out = np.concatenate([res.results[i]["y"] for i in range(NCORES)],
                         axis=0).astype(np.float32)
    if _trace:
        _cache['last_exec_time_ns'] = res.exec_time_ns
    return out
